# revision 18
# baseline (speedup 1.0000x reference)
"""Trainium2 Bass kernel for nn_BboxInteractionNetwork.

See reference: per row r of N = 32*512*3 rows (4 objects x 4 coords each):
  a0 = relu(x0 @ sa0_w + sa0_b)             ; sa = a0 @ sa1_w + sa1_b + a0
  b0_o = relu(x_o @ sb0_w + sb0_b), o=1..3  ; sb_o = b0_o @ sb1_w + sb1_b + b0_o
  r0_ij = relu(x_i @ rel0_w[:4] + x_j @ rel0_w[4:] + rel0_b)   (12 ordered pairs)
  rel_dyn_i = sum_j r0_ij @ rel1_w + rel1_b  summed over the 3 pairs w/ sender i
  dyn = self_dyn + rel_dyn                  ; f0 = relu(dyn @ aff0_w + aff0_b)
  f1 = f0 @ aff1_w + aff1_b + f0            ; final = f1.mean(objects)

Algebraic restructuring (exact in f32):
  - scatter-sum commutes with the linear rel1 layer:
      rel_dyn_i = (sum_j r0_ij) @ (rel1_w + I) + 3*rel1_b
  - residuals fold into weights:  h @ W + b + h == h @ (W + I) + b
  - the final mean commutes with aff1: final = (0.25*sum_o f0_o)@(aff1_w+I)+aff1_b
  - all layer-1 matmuls read one transposed coordinate tile Xt[16, F] through
    zero-padded [16,128] weight blocks; biases ride the PSUM->SBUF relu/copy
    as per-partition vectors (feature-major layout).

Layout: feature-major streaming ([128 features, F rows] tiles). The final
layer uses the mean tile itself as lhsT which yields row-major [rows, feats]
output, DMA'd contiguously without a transpose.

The per-tile dependency chain (L1 -> relu -> sums -> dyn -> copy -> aff0 ->
f0 -> mean -> final) is software-pipelined across 4 skewed stages so the
TensorEngine never waits on the pointwise engines; per-sender sums and the
mean tree run on GPSIMD to unload DVE/ACT.

Sharding: pure data parallel over 8 NeuronCores (batch dim); weights
replicated; host does the cheap weight fold/pad preprocessing.
"""

import numpy as np
import ml_dtypes

import concourse.bacc as bacc
import concourse.mybir as mybir
import concourse.tile as tile
from concourse.bass_utils import run_bass_kernel_spmd
from bass_rust import add_dep_helper

DT = mybir.dt
AF = mybir.ActivationFunctionType
ALU = mybir.AluOpType

B, T, C, E = 32, 512, 3, 128
NCORES = 8
ROWS = B * T * C // NCORES       # 6144 N-rows per core
F = 256                          # rows per tile iteration
NT = ROWS // F                   # 24 tile iterations
NBLK = F // 128                  # 2 row-blocks of 128 per tile

_PAIRS = [(i, j) for i in range(4) for j in range(4) if i != j]

_cache = {}


def _build():
    nc = bacc.Bacc("TRN2", target_bir_lowering=False, debug=False)

    xin = nc.dram_tensor("xin", [ROWS, 16], DT.float32, kind="ExternalInput")
    wl1 = nc.dram_tensor("wl1", [128, 8 * 128], DT.bfloat16, kind="ExternalInput")
    w2 = nc.dram_tensor("w2", [128, 5 * 128], DT.bfloat16, kind="ExternalInput")
    bia = nc.dram_tensor("bia", [128, 8], DT.float32, kind="ExternalInput")
    bc2 = nc.dram_tensor("bc2", [128, F], DT.float32, kind="ExternalInput")
    idn = nc.dram_tensor("idn", [128, 128], DT.bfloat16, kind="ExternalInput")
    out = nc.dram_tensor("out", [ROWS, 128], DT.float32, kind="ExternalOutput")

    x_v = xin.rearrange("(b p) c -> p b c", p=128)            # [128, 48, 16]
    out_v = out.rearrange("(t b p) e -> t p b e", b=NBLK, p=128)

    with tile.TileContext(nc) as tc:
        with (
            tc.tile_pool(name="const", bufs=1) as cp,
            tc.tile_pool(name="sb", bufs=3) as sb,
            tc.tile_pool(name="ps4", bufs=2, space="PSUM") as ps4,
            tc.tile_pool(name="psdyn", bufs=1, space="PSUM") as psdyn,
            tc.tile_pool(name="psmisc", bufs=2, space="PSUM") as psmisc,
        ):
            # ---- constants ----
            wl1_s = cp.tile([128, 8 * 128], DT.bfloat16, name="wl1_s")
            nc.sync.dma_start(out=wl1_s[:, :], in_=wl1[:, :])
            w2_s = cp.tile([128, 5 * 128], DT.bfloat16, name="w2_s")
            nc.sync.dma_start(out=w2_s[:, :], in_=w2[:, :])
            bia_s = cp.tile([128, 8], DT.float32, name="bia_s")
            nc.sync.dma_start(out=bia_s[:, :], in_=bia[:, :])
            bc2_s = cp.tile([128, F], DT.float32, name="bc2_s")
            nc.sync.dma_start(out=bc2_s[:, :], in_=bc2[:, :])
            idn_s = cp.tile([128, 128], DT.bfloat16, name="idn_s")
            nc.sync.dma_start(out=idn_s[:, :], in_=idn[:, :])
            # prologue (per quarter, so tile 0 starts early): DMA x, cast to
            # bf16, transpose to feature-major, replicate to partition group 64
            NQ = 6
            QB = ROWS // 128 // NQ            # 128-row blocks per quarter
            x_sb = cp.tile([128, ROWS * 16 // 128], DT.float32, name="x_sb")
            x_sb_v = x_sb.rearrange("p (b c) -> p b c", c=16)
            xc = cp.tile([128, ROWS * 16 // 128], DT.bfloat16, name="xc")
            xtq = [cp.tile([128, ROWS // NQ], DT.bfloat16, name=f"xtq{q}")
                   for q in range(NQ)]
            for q in range(NQ):
                nc.sync.dma_start(out=x_sb_v[:, q * QB:(q + 1) * QB, :],
                                  in_=x_v[:, q * QB:(q + 1) * QB, :])
                nc.vector.tensor_copy(xc[:, q * QB * 16:(q + 1) * QB * 16],
                                      x_sb[:, q * QB * 16:(q + 1) * QB * 16])
                for c in range(QB // 4):
                    ptp = psmisc.tile([16, 512], DT.bfloat16, name="ptp",
                                      tag="misc")
                    for k in range(4):
                        b = q * QB + c * 4 + k
                        nc.tensor.transpose(
                            ptp[:, k * 128:(k + 1) * 128],
                            xc[:, b * 16:(b + 1) * 16], idn_s[:, :])
                    nc.vector.tensor_copy(
                        xtq[q][0:16, c * 512:(c + 1) * 512], ptp[:, :])
                nc.sync.dma_start(out=xtq[q][64:80, :], in_=xtq[q][0:16, :])

            W_SA1, W_SB1, W_REL1, W_AFF0, W_AFF1 = range(5)

            def w2b(k):
                return w2_s[:, k * 128:(k + 1) * 128]

            hself_t, r0_t, dyn_t, m_t = {}, {}, {}, {}

            def stage_a(t):
                """trans + L1 (self + pairs) + relus + per-sender sums."""
                TQ = NT // NQ
                def xts(p):
                    return xtq[t // TQ][64 * p:64 * p + 16,
                                        (t % TQ) * F:(t % TQ + 1) * F]

                # concurrent row-tile pairs (positions 0 and 64) write
                # different PSUM banks: slice = qq + 2*p
                pself = ps4.tile([128, 4 * F], DT.float32, name="pself", tag="ps4")
                for qq in range(2):
                    for p in range(2):
                        sl = qq + 2 * p
                        nc.tensor.matmul(
                            pself[:, sl * F:(sl + 1) * F],
                            wl1_s[64 * p:64 * p + 16, qq * 128:(qq + 1) * 128],
                            xts(p), start=True, stop=True,
                            tile_position=(64 * p, 0))
                hself = sb.tile([128, 4 * F], DT.bfloat16, name="hself")
                nc.vector.tensor_scalar(
                    out=hself[:, 0:F], in0=pself[:, 0:F],
                    scalar1=bia_s[:, 0:1], scalar2=0.0,
                    op0=ALU.add, op1=ALU.max)
                nc.scalar.activation(hself[:, F:4 * F], pself[:, F:4 * F],
                                     AF.Relu, bias=bia_s[:, 1:2])
                hself_t[t] = hself

                r0 = sb.tile([128, 12 * F], DT.bfloat16, name="r0")
                for w in range(3):
                    pp = ps4.tile([128, 4 * F], DT.float32, name="pp", tag="ps4")
                    for qq in range(2):
                        q = 2 + 2 * w + qq
                        for p in range(2):
                            sl = qq + 2 * p
                            nc.tensor.matmul(
                                pp[:, sl * F:(sl + 1) * F],
                                wl1_s[64 * p:64 * p + 16, q * 128:(q + 1) * 128],
                                xts(p), start=True, stop=True,
                                tile_position=(64 * p, 0))
                    # write psum slice s to r0 block sigma(s) so r0 ends up
                    # in plain pair order (sigma is its own inverse)
                    ppv = pp.rearrange("q (a b f) -> q a b f", a=2, b=2)
                    r0w = r0[:, w * 4 * F:(w + 1) * 4 * F]
                    r0wv = r0w.rearrange("q (b a f) -> q a b f", b=2, a=2)
                    if w < 2:
                        nc.scalar.activation(
                            r0wv[:, :, :, :], ppv[:, :, :, :],
                            AF.Relu, bias=bia_s[:, 2:3])
                    else:
                        nc.vector.tensor_scalar(
                            out=r0wv[:, :, :, :], in0=ppv[:, :, :, :],
                            scalar1=bia_s[:, 2:3], scalar2=0.0,
                            op0=ALU.add, op1=ALU.max)

                r0_t[t] = r0

            def stage_b(t):
                """dyn accumulation (psum): self + 3 rel matmuls per sender."""
                hself, r0 = hself_t.pop(t), r0_t.pop(t)
                SIG = (0, 2, 1, 3)
                pdyn = psdyn.tile([128, 4 * F], DT.float32, name="pdyn")
                first_mm = [None, None]
                for i in range(4):
                    half, off = divmod(i, 2)
                    dst = pdyn[:, i * F:(i + 1) * F]
                    wsel = W_SA1 if i == 0 else W_SB1
                    hs = SIG[i]
                    mm = nc.tensor.matmul(
                        dst, w2b(wsel), hself[:, hs * F:(hs + 1) * F],
                        start=(off == 0), stop=False)
                    if off == 0:
                        first_mm[half] = mm
                    else:
                        add_dep_helper(mm.ins, first_mm[half].ins, sync=False,
                                       reason="bank has_written clear order")
                    for k in range(3):
                        p = 3 * i + k
                        nc.tensor.matmul(
                            dst, w2b(W_REL1), r0[:, p * F:(p + 1) * F],
                            start=False, stop=(i == 3 and k == 2))
                dyn = sb.tile([128, 4 * F], DT.bfloat16, name="dyn")
                nc.vector.tensor_scalar_add(dyn[:, 0:F], pdyn[:, 0:F],
                                            bia_s[:, 3:4])
                nc.scalar.activation(dyn[:, F:4 * F], pdyn[:, F:4 * F],
                                     AF.Identity, bias=bia_s[:, 4:5])
                dyn_t[t] = dyn

            def stage_c(t):
                """aff0 + relu + mean tree."""
                dyn = dyn_t.pop(t)
                pf0 = ps4.tile([128, 4 * F], DT.float32, name="pf0", tag="ps4")
                for i in range(4):
                    nc.tensor.matmul(
                        pf0[:, i * F:(i + 1) * F], w2b(W_AFF0),
                        dyn[:, i * F:(i + 1) * F], start=True, stop=True)
                f0 = sb.tile([128, 4 * F], DT.bfloat16, name="f0")
                nc.vector.tensor_scalar(
                    out=f0[:, :], in0=pf0[:, :],
                    scalar1=bia_s[:, 5:6], scalar2=0.0,
                    op0=ALU.add, op1=ALU.max)
                f0v = f0.rearrange("q (a b f) -> q b a f", a=2, b=2)
                t2 = sb.tile([128, 2 * F], DT.bfloat16, name="t2")
                t2v = t2.rearrange("q (a f) -> q a f", a=2)
                nc.gpsimd.tensor_add(t2v[:, :, :], f0v[:, 0], f0v[:, 1])
                m = sb.tile([128, F], DT.bfloat16, name="m")
                nc.gpsimd.tensor_add(m[:, :], t2[:, 0:F], t2[:, F:2 * F])
                m_t[t] = m

            def stage_d(t):
                """final matmuls (m as stationary) + bias add + DMA out."""
                m = m_t.pop(t)
                pout = psmisc.tile([128, F], DT.float32, name="pout", tag="misc")
                for blk in range(NBLK):
                    nc.tensor.matmul(
                        pout[:, blk * 128:(blk + 1) * 128],
                        m[:, blk * 128:(blk + 1) * 128],
                        w2b(W_AFF1), start=True, stop=True)
                outsb = sb.tile([128, F], DT.float32, name="outsb")
                nc.vector.tensor_add(outsb[:, :], pout[:, :], bc2_s[:, :])
                outsb_v = outsb.rearrange("p (b e) -> p b e", b=NBLK)
                nc.sync.dma_start(out=out_v[t], in_=outsb_v[:, :, :])

            for it in range(NT + 3):
                if 0 <= it - 1 < NT:
                    stage_b(it - 1)
                if 0 <= it - 2 < NT:
                    stage_c(it - 2)
                if 0 <= it - 3 < NT:
                    stage_d(it - 3)
                if it < NT:
                    stage_a(it)

    nc.compile()
    return nc


def _prep_inputs(inputs):
    f32 = np.float32
    bf16 = ml_dtypes.bfloat16
    I = np.eye(128, dtype=f32)

    # 2-way packed layout: row block [64p:64p+16] = weights for array row-tile
    # at partition base 64p; col block q = concurrent-pair index.
    # q=0,1: self matmuls (objects 2q+... item (q,p) -> object 2*q+p);
    # q=2+2w+qq: pair wave w, item (qq,p) -> pair 4w+2qq+p.
    wl1 = np.zeros((128, 8 * 128), f32)
    for q in range(2):
        for p in range(2):
            o = 2 * q + p
            wsel = inputs["sa0_w"] if o == 0 else inputs["sb0_w"]
            wl1[64 * p + 4 * o:64 * p + 4 * o + 4, q * 128:(q + 1) * 128] = wsel
    for w in range(3):
        for qq in range(2):
            q = 2 + 2 * w + qq
            for p in range(2):
                i, j = _PAIRS[4 * w + 2 * qq + p]
                r = 64 * p
                wl1[r + 4 * i:r + 4 * i + 4, q * 128:(q + 1) * 128] = inputs["rel0_w"][:4]
                wl1[r + 4 * j:r + 4 * j + 4, q * 128:(q + 1) * 128] += inputs["rel0_w"][4:]

    w2 = np.concatenate([
        inputs["sa1_w"] + I,
        inputs["sb1_w"] + I,
        inputs["rel1_w"] + I,
        inputs["aff0_w"],
        0.25 * (inputs["aff1_w"] + I),
    ], axis=1)

    bia = np.zeros((128, 8), f32)
    bia[:, 0] = inputs["sa0_b"]
    bia[:, 1] = inputs["sb0_b"]
    bia[:, 2] = inputs["rel0_b"]
    bia[:, 3] = inputs["sa1_b"] + 3.0 * inputs["rel1_b"]
    bia[:, 4] = inputs["sb1_b"] + 3.0 * inputs["rel1_b"]
    bia[:, 5] = inputs["aff0_b"]

    bc2 = np.tile(np.asarray(inputs["aff1_b"], f32), (128, NBLK))

    common = {
        "wl1": wl1.astype(bf16),
        "w2": np.asarray(w2, f32).astype(bf16),
        "bia": bia,
        "bc2": np.ascontiguousarray(bc2),
        "idn": I.astype(bf16),
    }
    x = np.asarray(inputs["x"], f32).reshape(NCORES, ROWS, 16)
    return [dict(common, xin=np.ascontiguousarray(x[c])) for c in range(NCORES)]


def _run(inputs, trace):
    inputs = {k: np.asarray(v) for k, v in inputs.items()}
    if "nc" not in _cache:
        _cache["nc"] = _build()
    nc = _cache["nc"]
    in_maps = _prep_inputs(inputs)
    res = run_bass_kernel_spmd(nc, in_maps, core_ids=list(range(NCORES)),
                               trace=trace)
    final = np.concatenate([r["out"] for r in res.results], axis=0)
    final = final.reshape(B, T, C * E)
    xf = inputs["x"].astype(np.float32).reshape(B * T * C, 4, 4)
    return (xf, final), res


def kernel(**inputs):
    out, _ = _run(inputs, trace=False)
    return out


def run_traced(**inputs):
    """Like kernel() but returns (output, BassKernelResults) with profiling."""
    return _run(inputs, trace=True)


# revision 19
# speedup vs baseline: 1.3588x; 1.3588x over previous
"""Trainium2 Bass kernel for nn_BboxInteractionNetwork.

See reference: per row r of N = 32*512*3 rows (4 objects x 4 coords each):
  a0 = relu(x0 @ sa0_w + sa0_b)             ; sa = a0 @ sa1_w + sa1_b + a0
  b0_o = relu(x_o @ sb0_w + sb0_b), o=1..3  ; sb_o = b0_o @ sb1_w + sb1_b + b0_o
  r0_ij = relu(x_i @ rel0_w[:4] + x_j @ rel0_w[4:] + rel0_b)   (12 ordered pairs)
  rel_dyn_i = sum_j r0_ij @ rel1_w + rel1_b  summed over the 3 pairs w/ sender i
  dyn = self_dyn + rel_dyn                  ; f0 = relu(dyn @ aff0_w + aff0_b)
  f1 = f0 @ aff1_w + aff1_b + f0            ; final = f1.mean(objects)

Algebraic restructuring (exact in f32):
  - scatter-sum commutes with the linear rel1 layer:
      rel_dyn_i = (sum_j r0_ij) @ (rel1_w + I) + 3*rel1_b
  - residuals fold into weights:  h @ W + b + h == h @ (W + I) + b
  - the final mean commutes with aff1: final = (0.25*sum_o f0_o)@(aff1_w+I)+aff1_b
  - all layer-1 matmuls read one transposed coordinate tile Xt[16, F] through
    zero-padded [16,128] weight blocks; biases ride the PSUM->SBUF relu/copy
    as per-partition vectors (feature-major layout).

Layout: feature-major streaming ([128 features, F rows] tiles). The final
layer uses the mean tile itself as lhsT which yields row-major [rows, feats]
output, DMA'd contiguously without a transpose.

The per-tile dependency chain (L1 -> relu -> sums -> dyn -> copy -> aff0 ->
f0 -> mean -> final) is software-pipelined across 4 skewed stages so the
TensorEngine never waits on the pointwise engines; per-sender sums and the
mean tree run on GPSIMD to unload DVE/ACT.

Sharding: pure data parallel over 8 NeuronCores (batch dim); weights
replicated; host does the cheap weight fold/pad preprocessing.
"""

import numpy as np
import ml_dtypes

import concourse.bacc as bacc
import concourse.mybir as mybir
import concourse.tile as tile
from concourse.bass_utils import run_bass_kernel_spmd
from bass_rust import add_dep_helper

DT = mybir.dt
AF = mybir.ActivationFunctionType
ALU = mybir.AluOpType

B, T, C, E = 32, 512, 3, 128
NCORES = 8
ROWS = B * T * C // NCORES       # 6144 N-rows per core
F = 256                          # rows per tile iteration
NT = ROWS // F                   # 24 tile iterations
NBLK = F // 128                  # 2 row-blocks of 128 per tile

_PAIRS = [(i, j) for i in range(4) for j in range(4) if i != j]

_cache = {}


def _build():
    nc = bacc.Bacc("TRN2", target_bir_lowering=False, debug=False)

    xin = nc.dram_tensor("xin", [ROWS, 16], DT.float32, kind="ExternalInput")
    wl1 = nc.dram_tensor("wl1", [128, 8 * 128], DT.bfloat16, kind="ExternalInput")
    w2 = nc.dram_tensor("w2", [128, 5 * 128], DT.bfloat16, kind="ExternalInput")
    bia = nc.dram_tensor("bia", [128, 8], DT.float32, kind="ExternalInput")
    bc2 = nc.dram_tensor("bc2", [128, F], DT.float32, kind="ExternalInput")
    idn = nc.dram_tensor("idn", [128, 128], DT.bfloat16, kind="ExternalInput")
    out = nc.dram_tensor("out", [ROWS, 128], DT.float32, kind="ExternalOutput")

    x_v = xin.rearrange("(b p) c -> p b c", p=128)            # [128, 48, 16]
    out_v = out.rearrange("(t b p) e -> t p b e", b=NBLK, p=128)

    with tile.TileContext(nc) as tc:
        with (
            tc.tile_pool(name="const", bufs=1) as cp,
            tc.tile_pool(name="sb", bufs=3) as sb,
            tc.tile_pool(name="ps4", bufs=2, space="PSUM") as ps4,
            tc.tile_pool(name="psdyn", bufs=1, space="PSUM") as psdyn,
            tc.tile_pool(name="psmisc", bufs=2, space="PSUM") as psmisc,
        ):
            # ---- constants ----
            wl1_s = cp.tile([128, 8 * 128], DT.bfloat16, name="wl1_s")
            nc.sync.dma_start(out=wl1_s[:, :], in_=wl1[:, :])
            w2_s = cp.tile([128, 5 * 128], DT.bfloat16, name="w2_s")
            nc.sync.dma_start(out=w2_s[:, :], in_=w2[:, :])
            bia_s = cp.tile([128, 8], DT.float32, name="bia_s")
            nc.sync.dma_start(out=bia_s[:, :], in_=bia[:, :])
            bc2_s = cp.tile([128, F], DT.float32, name="bc2_s")
            nc.sync.dma_start(out=bc2_s[:, :], in_=bc2[:, :])
            idn_s = cp.tile([128, 128], DT.bfloat16, name="idn_s")
            nc.sync.dma_start(out=idn_s[:, :], in_=idn[:, :])
            # prologue (per quarter, so tile 0 starts early): DMA x, cast to
            # bf16, transpose to feature-major, replicate to partition group 64
            NQ = 6
            QB = ROWS // 128 // NQ            # 128-row blocks per quarter
            x_sb = cp.tile([128, ROWS * 16 // 128], DT.float32, name="x_sb")
            x_sb_v = x_sb.rearrange("p (b c) -> p b c", c=16)
            xc = cp.tile([128, ROWS * 16 // 128], DT.bfloat16, name="xc")
            xtq = [cp.tile([128, ROWS // NQ], DT.bfloat16, name=f"xtq{q}")
                   for q in range(NQ)]
            for q in range(NQ):
                nc.sync.dma_start(out=x_sb_v[:, q * QB:(q + 1) * QB, :],
                                  in_=x_v[:, q * QB:(q + 1) * QB, :])
                nc.vector.tensor_copy(xc[:, q * QB * 16:(q + 1) * QB * 16],
                                      x_sb[:, q * QB * 16:(q + 1) * QB * 16])
                for c in range(QB // 4):
                    ptp = psmisc.tile([16, 512], DT.bfloat16, name="ptp",
                                      tag="misc")
                    for k in range(4):
                        b = q * QB + c * 4 + k
                        nc.tensor.transpose(
                            ptp[:, k * 128:(k + 1) * 128],
                            xc[:, b * 16:(b + 1) * 16], idn_s[:, :])
                    nc.vector.tensor_copy(
                        xtq[q][0:16, c * 512:(c + 1) * 512], ptp[:, :])
                nc.sync.dma_start(out=xtq[q][64:80, :], in_=xtq[q][0:16, :])

            W_SA1, W_SB1, W_REL1, W_AFF0, W_AFF1 = range(5)

            def w2b(k):
                return w2_s[:, k * 128:(k + 1) * 128]

            hself_t, r0_t, dyn_t, m_t = {}, {}, {}, {}

            def stage_a(t):
                """trans + L1 (self + pairs) + relus + per-sender sums."""
                TQ = NT // NQ
                def xts(p):
                    return xtq[t // TQ][64 * p:64 * p + 16,
                                        (t % TQ) * F:(t % TQ + 1) * F]

                # concurrent row-tile pairs (positions 0 and 64) write
                # different PSUM banks: slice = qq + 2*p
                pself = ps4.tile([128, 4 * F], DT.float32, name="pself", tag="ps4")
                for qq in range(2):
                    for p in range(2):
                        sl = qq + 2 * p
                        nc.tensor.matmul(
                            pself[:, sl * F:(sl + 1) * F],
                            wl1_s[64 * p:64 * p + 16, qq * 128:(qq + 1) * 128],
                            xts(p), start=True, stop=True,
                            tile_position=(64 * p, 0))
                hself = sb.tile([128, 4 * F], DT.bfloat16, name="hself")
                nc.vector.tensor_scalar(
                    out=hself[:, 0:F], in0=pself[:, 0:F],
                    scalar1=bia_s[:, 0:1], scalar2=0.0,
                    op0=ALU.add, op1=ALU.max)
                nc.scalar.activation(hself[:, F:4 * F], pself[:, F:4 * F],
                                     AF.Relu, bias=bia_s[:, 1:2])
                hself_t[t] = hself

                r0 = sb.tile([128, 12 * F], DT.bfloat16, name="r0")
                for w in range(3):
                    pp = ps4.tile([128, 4 * F], DT.float32, name="pp", tag="ps4")
                    for qq in range(2):
                        q = 2 + 2 * w + qq
                        for p in range(2):
                            sl = qq + 2 * p
                            nc.tensor.matmul(
                                pp[:, sl * F:(sl + 1) * F],
                                wl1_s[64 * p:64 * p + 16, q * 128:(q + 1) * 128],
                                xts(p), start=True, stop=True,
                                tile_position=(64 * p, 0))
                    # write psum slice s to r0 block sigma(s) so r0 ends up
                    # in plain pair order (sigma is its own inverse)
                    ppv = pp.rearrange("q (a b f) -> q a b f", a=2, b=2)
                    r0w = r0[:, w * 4 * F:(w + 1) * 4 * F]
                    r0wv = r0w.rearrange("q (b a f) -> q a b f", b=2, a=2)
                    if w < 2:
                        nc.scalar.activation(
                            r0wv[:, :, :, :], ppv[:, :, :, :],
                            AF.Relu, bias=bia_s[:, 2:3])
                    else:
                        nc.vector.tensor_scalar(
                            out=r0wv[:, :, :, :], in0=ppv[:, :, :, :],
                            scalar1=bia_s[:, 2:3], scalar2=0.0,
                            op0=ALU.add, op1=ALU.max)

                r0_t[t] = r0

            def stage_b(t):
                """dyn accumulation (psum): self + 3 rel matmuls per sender."""
                hself, r0 = hself_t.pop(t), r0_t.pop(t)
                SIG = (0, 2, 1, 3)
                pdyn = psdyn.tile([128, 4 * F], DT.float32, name="pdyn")
                first_mm = [None, None]
                for i in range(4):
                    half, off = divmod(i, 2)
                    dst = pdyn[:, i * F:(i + 1) * F]
                    wsel = W_SA1 if i == 0 else W_SB1
                    hs = SIG[i]
                    mm = nc.tensor.matmul(
                        dst, w2b(wsel), hself[:, hs * F:(hs + 1) * F],
                        start=(off == 0), stop=False)
                    if off == 0:
                        first_mm[half] = mm
                    else:
                        add_dep_helper(mm.ins, first_mm[half].ins, sync=False,
                                       reason="bank has_written clear order")
                    for k in range(3):
                        p = 3 * i + k
                        nc.tensor.matmul(
                            dst, w2b(W_REL1), r0[:, p * F:(p + 1) * F],
                            start=False, stop=(i == 3 and k == 2))
                dyn = sb.tile([128, 4 * F], DT.bfloat16, name="dyn")
                nc.vector.tensor_scalar_add(dyn[:, 0:F], pdyn[:, 0:F],
                                            bia_s[:, 3:4])
                nc.scalar.activation(dyn[:, F:4 * F], pdyn[:, F:4 * F],
                                     AF.Identity, bias=bia_s[:, 4:5])
                dyn_t[t] = dyn

            def stage_c(t):
                """aff0 + relu + mean tree."""
                dyn = dyn_t.pop(t)
                pf0 = ps4.tile([128, 4 * F], DT.float32, name="pf0", tag="ps4")
                for i in range(4):
                    nc.tensor.matmul(
                        pf0[:, i * F:(i + 1) * F], w2b(W_AFF0),
                        dyn[:, i * F:(i + 1) * F], start=True, stop=True)
                f0 = sb.tile([128, 4 * F], DT.bfloat16, name="f0")
                nc.vector.tensor_scalar(
                    out=f0[:, :], in0=pf0[:, :],
                    scalar1=bia_s[:, 5:6], scalar2=0.0,
                    op0=ALU.add, op1=ALU.max)
                f0v = f0.rearrange("q (a b f) -> q b a f", a=2, b=2)
                t2 = sb.tile([128, 2 * F], DT.bfloat16, name="t2")
                t2v = t2.rearrange("q (a f) -> q a f", a=2)
                nc.gpsimd.tensor_add(t2v[:, :, :], f0v[:, 0], f0v[:, 1])
                m = sb.tile([128, F], DT.bfloat16, name="m")
                nc.gpsimd.tensor_add(m[:, :], t2[:, 0:F], t2[:, F:2 * F])
                m_t[t] = m

            def stage_d(t):
                """final matmuls (m as stationary) + bias add + DMA out."""
                m = m_t.pop(t)
                pout = psmisc.tile([128, F], DT.float32, name="pout", tag="misc")
                for blk in range(NBLK):
                    nc.tensor.matmul(
                        pout[:, blk * 128:(blk + 1) * 128],
                        m[:, blk * 128:(blk + 1) * 128],
                        w2b(W_AFF1), start=True, stop=True)
                outsb = sb.tile([128, F], DT.float32, name="outsb")
                nc.vector.tensor_add(outsb[:, :], pout[:, :], bc2_s[:, :])
                outsb_v = outsb.rearrange("p (b e) -> p b e", b=NBLK)
                nc.sync.dma_start(out=out_v[t], in_=outsb_v[:, :, :])

            for it in range(NT + 3):
                if it < NT:
                    stage_a(it)
                if 0 <= it - 1 < NT:
                    stage_b(it - 1)
                if 0 <= it - 2 < NT:
                    stage_c(it - 2)
                if 0 <= it - 3 < NT:
                    stage_d(it - 3)

    nc.compile()
    return nc


def _prep_inputs(inputs):
    f32 = np.float32
    bf16 = ml_dtypes.bfloat16
    I = np.eye(128, dtype=f32)

    # 2-way packed layout: row block [64p:64p+16] = weights for array row-tile
    # at partition base 64p; col block q = concurrent-pair index.
    # q=0,1: self matmuls (objects 2q+... item (q,p) -> object 2*q+p);
    # q=2+2w+qq: pair wave w, item (qq,p) -> pair 4w+2qq+p.
    wl1 = np.zeros((128, 8 * 128), f32)
    for q in range(2):
        for p in range(2):
            o = 2 * q + p
            wsel = inputs["sa0_w"] if o == 0 else inputs["sb0_w"]
            wl1[64 * p + 4 * o:64 * p + 4 * o + 4, q * 128:(q + 1) * 128] = wsel
    for w in range(3):
        for qq in range(2):
            q = 2 + 2 * w + qq
            for p in range(2):
                i, j = _PAIRS[4 * w + 2 * qq + p]
                r = 64 * p
                wl1[r + 4 * i:r + 4 * i + 4, q * 128:(q + 1) * 128] = inputs["rel0_w"][:4]
                wl1[r + 4 * j:r + 4 * j + 4, q * 128:(q + 1) * 128] += inputs["rel0_w"][4:]

    w2 = np.concatenate([
        inputs["sa1_w"] + I,
        inputs["sb1_w"] + I,
        inputs["rel1_w"] + I,
        inputs["aff0_w"],
        0.25 * (inputs["aff1_w"] + I),
    ], axis=1)

    bia = np.zeros((128, 8), f32)
    bia[:, 0] = inputs["sa0_b"]
    bia[:, 1] = inputs["sb0_b"]
    bia[:, 2] = inputs["rel0_b"]
    bia[:, 3] = inputs["sa1_b"] + 3.0 * inputs["rel1_b"]
    bia[:, 4] = inputs["sb1_b"] + 3.0 * inputs["rel1_b"]
    bia[:, 5] = inputs["aff0_b"]

    bc2 = np.tile(np.asarray(inputs["aff1_b"], f32), (128, NBLK))

    common = {
        "wl1": wl1.astype(bf16),
        "w2": np.asarray(w2, f32).astype(bf16),
        "bia": bia,
        "bc2": np.ascontiguousarray(bc2),
        "idn": I.astype(bf16),
    }
    x = np.asarray(inputs["x"], f32).reshape(NCORES, ROWS, 16)
    return [dict(common, xin=np.ascontiguousarray(x[c])) for c in range(NCORES)]


def _run(inputs, trace):
    inputs = {k: np.asarray(v) for k, v in inputs.items()}
    if "nc" not in _cache:
        _cache["nc"] = _build()
    nc = _cache["nc"]
    in_maps = _prep_inputs(inputs)
    res = run_bass_kernel_spmd(nc, in_maps, core_ids=list(range(NCORES)),
                               trace=trace)
    final = np.concatenate([r["out"] for r in res.results], axis=0)
    final = final.reshape(B, T, C * E)
    xf = inputs["x"].astype(np.float32).reshape(B * T * C, 4, 4)
    return (xf, final), res


def kernel(**inputs):
    out, _ = _run(inputs, trace=False)
    return out


def run_traced(**inputs):
    """Like kernel() but returns (output, BassKernelResults) with profiling."""
    return _run(inputs, trace=True)
